# revision 17
# baseline (speedup 1.0000x reference)
"""Trainium2 Bass kernel for nn_CrossAttention_82429012345074.

8-head self-attention, B=2, N=4096, d_model=512, 8 heads x 64 dim.

Sharding: one head per NeuronCore (8 heads / 8 cores) — tensor parallel.
Host sums the 8 per-head partial outputs (each divided by its softmax
denominator, which the device ships separately) and adds the bias.

Per-core device kernel (bf16 matmuls, fp32 accumulation):
  phase A   : per 512-token slab, qk projection as ONE matmul with the
              stacked [Wq;Wk] stationary operand (full 128-col array), and
              v projection into token-partition layout.
  attention : per 512-query group, 11 score chunks of <=3 key blocks.
              Score matmuls are ROW-PAIRED with tile_position — the PE
              runs as 2x (64x128) tiles, two key blocks per 512-cycle
              slot (contraction is only 64).  exp() on ScalarE (the
              critical path: ~262us floor).  PV accumulates unnormalized
              output + denominator row (ones column in v).  Output
              projection (also K=64, row-tiled T0) emits the UNNORMALIZED
              out partial; the denominator is shipped to the host, which
              divides (linearity: (o/den)@Wo == (o@Wo)/den).
  The next batch's projections are interleaved into the attention loop's
  PE idle slots (keeps the PE HAM clock-gate warm with useful work).
"""

import sys

sys.path.insert(0, "/opt/trn_rl_repo")

import numpy as np
import ml_dtypes

B, N, D, H, DH = 2, 4096, 512, 8, 64
TOK = B * N            # 8192
NQ = 512               # query-group width
NCH = D // 128         # 4 feature chunks of x
NJB = N // 128         # 32 key blocks per batch
NTB = TOK // 128       # 64 token blocks
NSLAB = TOK // NQ      # 16 token slabs (8 per batch)
CHUNKS = [3] * 10 + [2]   # key-blocks per exp() chunk (sum = 32)
SCALE = DH ** -0.5
N_FILL = 1             # junk matmuls per score pair (HAM warm-pinning)


def build_bass():
    from contextlib import ExitStack

    import concourse.bass as bass
    import concourse.mybir as mybir
    import concourse.tile as tile
    from concourse import bacc

    f32 = mybir.dt.float32
    bf16 = mybir.dt.bfloat16
    fp8 = mybir.dt.float8e4
    DR = mybir.MatmulPerfMode.DoubleRow
    EXP = mybir.ActivationFunctionType.Exp

    nc = bacc.Bacc("TRN2", target_bir_lowering=False, num_devices=8)
    x_d = nc.dram_tensor("x", [NCH, 128, TOK], bf16, kind="ExternalInput")
    wqk_d = nc.dram_tensor("wqk", [D, 128], bf16, kind="ExternalInput")
    wv_d = nc.dram_tensor("wv", [D, DH], bf16, kind="ExternalInput")
    wo_d = nc.dram_tensor("wo", [DH, D], bf16, kind="ExternalInput")
    out_d = nc.dram_tensor("out", [TOK, D], f32, kind="ExternalOutput")
    den_d = nc.dram_tensor("den", [NSLAB, NQ], f32, kind="ExternalOutput")

    with tile.TileContext(nc) as tc, ExitStack() as ctx:
        const = ctx.enter_context(tc.tile_pool(name="const", bufs=1))
        sb_p = ctx.enter_context(tc.tile_pool(name="sb_p", bufs=3))
        sb_io = ctx.enter_context(tc.tile_pool(name="sb_io", bufs=3))
        ps_s = ctx.enter_context(tc.tile_pool(name="ps_s", bufs=2, space="PSUM"))
        ps_o = ctx.enter_context(tc.tile_pool(name="ps_o", bufs=1, space="PSUM"))
        ps_fp = ctx.enter_context(tc.tile_pool(name="ps_fp", bufs=1, space="PSUM"))

        # Long-lived SBUF tensors
        xT = const.tile([128, NCH, TOK], bf16, name="xT")      # x^T, 4 chunks
        qkB = const.tile([128, TOK], bf16, name="qkB")         # [q; k] per token
        qkA = const.tile([128, TOK], bf16, name="qkA")         # [k; q] per token
        # v in fp8, paired key blocks for DoubleRow PV: [pair, 2, 80(pad)]
        vP = const.tile([128, NTB // 2, 2, 80], fp8, name="vP")
        # exp(scores) ring, fp8, 6 one-block slots (chunks alternate halves;
        # DoubleRow PV reads adjacent slot pairs)
        pR = const.tile([128, 6, NQ], fp8, name="pR")
        oNd = const.tile([64, TOK], bf16, name="oNd")          # unnorm attn out^T
        wqk = const.tile([128, NCH, 128], bf16, name="wqk")
        wv = const.tile([128, NCH, DH], bf16, name="wv")
        wo = const.tile([64, D], bf16, name="wo")

        nc.vector.memset(vP[:, :, :, DH : DH + 1], 1.0)
        vP_blk = vP.rearrange("p np two d -> p (np two) d")    # [128, NTB, 80]

        nc.sync.dma_start(out=wqk, in_=wqk_d[:].rearrange("(c p) m -> p c m", p=128))
        nc.sync.dma_start(out=wv, in_=wv_d[:].rearrange("(c p) d -> p c d", p=128))
        nc.sync.dma_start(out=wo, in_=wo_d[:])

        # Preload the exp activation-table set (hide the ~2.7us load in phase A)
        warm_s = sb_io.tile([1, 1], f32, name="warm_s")
        warm_p = sb_io.tile([1, 1], f32, name="warm_p")
        nc.vector.memset(warm_s, 0.0)
        nc.scalar.activation(out=warm_p, in_=warm_s, func=EXP, scale=1.0)

        # x^T slab DMAs (per 512-token slab, per feature chunk)
        for s in range(NSLAB):
            t0 = s * NQ
            for c in range(NCH):
                nc.sync.dma_start(
                    out=xT[:, c, t0 : t0 + NQ], in_=x_d[c, :, t0 : t0 + NQ]
                )

        def emit_proj_slab(s, qk_psum, vp_psum):
            """Projections for one 512-token slab.

            qk_psum: [128, 512] bank — q rows 0-63, k rows 64-127
            vp_psum: [128, 256] region — 4 token blocks x 64 v-dims
            """
            t0 = s * NQ
            if qk_psum is not None:
                for c in range(NCH):
                    nc.tensor.matmul(
                        qk_psum, lhsT=wqk[:, c, :], rhs=xT[:, c, t0 : t0 + NQ],
                        start=(c == 0), stop=(c == NCH - 1),
                    )
            if vp_psum is not None:
                for t in range(4):
                    tb0 = t0 + t * 128
                    for c in range(NCH):
                        nc.tensor.matmul(
                            vp_psum[:, t * 64 : t * 64 + 64],
                            lhsT=xT[:, c, tb0 : tb0 + 128], rhs=wv[:, c, :],
                            start=(c == 0), stop=(c == NCH - 1),
                        )

        def emit_proj_copies(s, qk_psum, vp_psum):
            t0 = s * NQ
            nc.vector.tensor_copy(out=qkB[:, t0 : t0 + NQ], in_=qk_psum)
            nc.vector.tensor_copy(
                out=vP_blk[:, s * 4 : s * 4 + 4, 0:DH],
                in_=vp_psum.rearrange("p (t d) -> p t d", t=4),
            )
            # build qkA = [k; q] from qkB = [q; k] (partition swap via DMA)
            nc.sync.dma_start(
                out=qkA[0:64, t0 : t0 + NQ], in_=qkB[64:128, t0 : t0 + NQ]
            )
            nc.sync.dma_start(
                out=qkA[64:128, t0 : t0 + NQ], in_=qkB[0:64, t0 : t0 + NQ]
            )

        # ---- Phase A: batch-0 projections (slabs 0-7) ----
        for s in range(B * 4):
            sA = ps_s.tile([128, 3, NQ], f32, tag="s", name="sA")
            emit_proj_slab(s, sA[:, 0, :], sA[:, 1, 0:256])
            emit_proj_copies(s, sA[:, 0, :], sA[:, 1, 0:256])

        # ---- Attention main loop ----
        # Cycle c of each group: (64,128)-mode window [score pairs + out-proj
        # of the previous group + fillers], exp(c) on ScalarE, (128,128)-mode
        # window [PV(c-1) + interleaved next-batch projections].
        CSTART = [0, 3, 6, 9, 12, 15, 18, 21, 24, 27, 30]  # chunk -> first block
        PAIRS_OF_CYCLE = [
            [0, 1], [2], [3, 4], [5], [6, 7], [8],
            [9, 10], [11], [12, 13], [14], [15],
        ]

        pending = None  # (q0,) of the previous group awaiting out-projection

        def emit_fp_one(pq0, t):
            tt0 = pq0 + t * 128
            fp = ps_fp.tile([128, D], f32, tag="fp", name="fp")
            nc.tensor.matmul(
                fp, lhsT=oNd[:, tt0 : tt0 + 128], rhs=wo,
                start=True, stop=True, tile_position=(0, 0),
            )
            ob = sb_io.tile([128, D], f32, tag="ob", name="ob")
            nc.vector.tensor_copy(out=ob, in_=fp)
            nc.sync.dma_start(out=out_d[tt0 : tt0 + 128, :], in_=ob)

        for g in range(2 * NSLAB // 2):  # 16 groups
            b, qg = g // 8, g % 8
            q0 = b * N + qg * NQ
            o = ps_o.tile([128, NQ], f32, tag="o", name="o")
            s_tiles = {}

            def s_slice(blk):
                c, i = blk // 3, blk % 3
                if c not in s_tiles:
                    s_tiles[c] = ps_s.tile([128, 3, NQ], f32, tag="s", name="s")
                return s_tiles[c][:, i, :]

            # b1 projection slab interleaved into groups 0..7 (all of
            # batch 1's q/k/v ready before batch-1 attention starts)
            proj_slab = 8 + g if g < 8 else None

            def flush_pv(upto):
                # emit PV DoubleRow pairs [flush_pv.done, upto)
                for p_i2 in range(flush_pv.done, upto):
                    nc.tensor.matmul(
                        o[0 : DH + 1, :],
                        lhsT=vP[:, b * (NJB // 2) + p_i2, :, 0 : DH + 1],
                        rhs=pR[:, (2 * p_i2) % 6 : (2 * p_i2) % 6 + 2, :],
                        start=(p_i2 == 0), stop=(p_i2 == NJB // 2 - 1),
                        perf_mode=DR,
                    )
                flush_pv.done = upto

            flush_pv.done = 0

            for c in range(11):
                # ---- (64,128)-mode window: score pairs ----
                for p_i in PAIRS_OF_CYCLE[c]:
                    b0, b1_ = 2 * p_i, 2 * p_i + 1
                    s0 = s_slice(b0)
                    j0 = b * N + b0 * 128
                    # HAM filler: junk matmuls into the slice the real pair
                    # overwrites (start=True clears them)
                    for _f in range(N_FILL):
                        nc.tensor.matmul(
                            s0[:, 0:64], lhsT=qkA[0:64, 0:128],
                            rhs=qkB[0:64, 0:64],
                            start=True, stop=True, tile_position=(0, 0),
                        )
                    nc.tensor.matmul(
                        s0, lhsT=qkA[0:64, j0 : j0 + 128],
                        rhs=qkB[0:64, q0 : q0 + NQ],
                        start=True, stop=True, tile_position=(0, 0),
                    )
                    if b1_ < NJB:
                        s1 = s_slice(b1_)
                        j1 = b * N + b1_ * 128
                        nc.tensor.matmul(
                            s1, lhsT=qkB[64:128, j1 : j1 + 128],
                            rhs=qkA[64:128, q0 : q0 + NQ],
                            start=True, stop=True, tile_position=(64, 0),
                        )
                # proj-psum evacuation copies FIRST (so the fp-slot WAR chain
                # on the DVE queue stays acyclic), then the out-projection of
                # the previous group.
                if proj_slab is not None:
                    if c == 6:
                        nc.vector.tensor_copy(
                            out=qkB[:, proj_slab * NQ : proj_slab * NQ + NQ],
                            in_=qk_ps,
                        )
                    elif c == 8:
                        t0p = proj_slab * NQ
                        nc.vector.tensor_copy(
                            out=vP_blk[:, proj_slab * 4 : proj_slab * 4 + 4, 0:DH],
                            in_=vp_ps.rearrange("p (t d) -> p t d", t=4),
                        )
                        nc.sync.dma_start(
                            out=qkA[0:64, t0p : t0p + NQ],
                            in_=qkB[64:128, t0p : t0p + NQ],
                        )
                        nc.sync.dma_start(
                            out=qkA[64:128, t0p : t0p + NQ],
                            in_=qkB[0:64, t0p : t0p + NQ],
                        )
                if pending is not None and c in (2, 4, 6, 8):
                    emit_fp_one(pending, (c - 2) // 2)

                # ---- deferred PV (DoubleRow pairs), BEFORE exp(c) so the
                # ring WAR/RAW ordering stays correct ----
                if c > 0:
                    # pairs fully covered by chunks <= c-1 (blocks <= 3c-1)
                    flush_pv((3 * c - 2) // 2 + 1)

                # ---- exp(c) on ScalarE, fp8 out into the ring ----
                gsz = CHUNKS[c]
                off = (3 * c) % 6
                nc.scalar.activation(
                    out=pR[:, off : off + gsz, :], in_=s_tiles[c][:, 0:gsz, :],
                    func=EXP, scale=SCALE,
                )

                # ---- (128,128)-mode window: next-batch projections ----
                if proj_slab is not None:
                    if c == 4:
                        qk_ps = ps_fp.tile([128, NQ], f32, tag="fp", name="qk_ps")
                        emit_proj_slab(proj_slab, qk_ps, None)
                    elif c == 6:
                        vp_ps = ps_fp.tile([128, 256], f32, tag="fp", name="vp_ps")
                        emit_proj_slab(proj_slab, None, vp_ps)

            flush_pv(NJB // 2)

            # group epilogue: unnormalized attn out (bf16) + denominator (f32)
            nc.vector.tensor_copy(out=oNd[:, q0 : q0 + NQ], in_=o[0:DH, :])
            denb = sb_io.tile([128, NQ], f32, tag="den", name="denb")
            nc.vector.tensor_copy(out=denb[DH : DH + 1, :], in_=o[DH : DH + 1, :])
            nc.sync.dma_start(
                out=den_d[g, :].unsqueeze(0), in_=denb[DH : DH + 1, :]
            )
            pending = q0

        for t in range(4):
            emit_fp_one(pending, t)

    nc.compile()
    return nc


def make_in_maps(x, Wq, Wk, Wv, Wo):
    bf16 = ml_dtypes.bfloat16
    x_bf = np.ascontiguousarray(
        x.reshape(TOK, D).T.reshape(NCH, 128, TOK)
    ).astype(bf16)
    in_maps = []
    for h in range(H):
        sl = slice(h * DH, (h + 1) * DH)
        wqk = np.concatenate([Wq[sl, :].T, Wk[sl, :].T], axis=1)  # [512, 128]
        in_maps.append(
            {
                "x": x_bf,
                "wqk": np.ascontiguousarray(wqk).astype(bf16),
                "wv": np.ascontiguousarray(Wv[sl, :].T).astype(bf16),
                "wo": np.ascontiguousarray(Wo[:, sl].T).astype(bf16),
            }
        )
    return in_maps


def _install_ntff_shim():
    """The axon boot skips registering the NTFF profile hook when the image's
    antenv lacks axon_hooks; register an equivalent shim so trace=True works."""
    import types

    if "antenv.axon_hooks" in sys.modules:
        return
    try:
        from trn_agent_boot.trn_boot import _ntff_profile_via_ctypes

        hook = _ntff_profile_via_ctypes("/opt/axon/libaxon_pjrt.so")
    except Exception:
        hook = None
    mod = types.ModuleType("antenv.axon_hooks")
    mod.get_axon_ntff_profile_hook = lambda: hook
    sys.modules["antenv.axon_hooks"] = mod


def run(x, Wq, Wk, Wv, Wo, bo, trace=False):
    from concourse.bass_utils import run_bass_kernel_spmd

    if trace:
        _install_ntff_shim()

    nc = build_bass()
    in_maps = make_in_maps(x, Wq, Wk, Wv, Wo)
    res = run_bass_kernel_spmd(nc, in_maps, core_ids=list(range(H)), trace=trace)
    acc = np.zeros((TOK, D), dtype=np.float32)
    for r in res.results:
        den = r["den"].reshape(TOK, 1)
        acc += r["out"] / den
    acc += np.asarray(bo, dtype=np.float32)[None, :]
    return acc.reshape(B, N, D), res


def kernel(x, Wq, Wk, Wv, Wo, bo):
    out, _ = run(
        np.asarray(x, dtype=np.float32),
        np.asarray(Wq, dtype=np.float32),
        np.asarray(Wk, dtype=np.float32),
        np.asarray(Wv, dtype=np.float32),
        np.asarray(Wo, dtype=np.float32),
        np.asarray(bo, dtype=np.float32),
    )
    return out
